# revision 5
# baseline (speedup 1.0000x reference)
"""Trainium2 Bass kernel for nn_Net_4200478015619 (dense_mlp).

Computes, for x (262144, 128) fp32 and W (100, 128) fp32:
    z   = x @ W.T                      # (B, 100)
    y   = z**3 + 0.1 * z
    out = sum(y, axis=1, keepdims=True)  # (B, 1)

Sharding: pure data parallel over 8 NeuronCores — core c gets rows
[c*32768, (c+1)*32768). Each shard is transposed on the host to
xT (128, 32768) so the feature dim lands on SBUF partitions and the
matmul contraction needs no on-chip transpose.

Per-core dataflow (Tile-scheduled pipeline over 64 chunks of 512 rows):
    DMA    : xT super-tiles [128, 2048] -> SBUF (float32r)
    PE MM1 : zT [100, 512] = W @ xT_chunk      (fp32r, ~1 cyc/col)
    ACT    : s = Square(zT)          PSUM -> SBUF
    DVE STT: y = (s + 0.1) * zT      (scalar_tensor_tensor, fused)
    PE MM2 : out row = ones.T @ y    -> PSUM partition 32*j (tile_position
             col-tiling packs 4 chunks into one PSUM bank)
    ACT    : strided copy [4, 512] PSUM -> SBUF (once per 4 chunks)
    DMA    : [4, 512] SBUF -> DRAM
"""

import numpy as np

import concourse.bacc as bacc
import concourse.mybir as mybir
import concourse.tile as tile
from concourse.bass_utils import run_bass_kernel_spmd

N_CORES = 8
B = 262144
B_CORE = B // N_CORES  # 32768
F = 128
M = 100
ALPHA = 0.1
CHUNK = 512
SUPER = 2048
N_SUPER = B_CORE // SUPER  # 16
CHUNKS_PER_SUPER = SUPER // CHUNK  # 4
N_CHUNKS = B_CORE // CHUNK  # 64
GROUP = 4  # chunks per output PSUM bank


def build_nc():
    nc = bacc.Bacc()
    xt = nc.declare_dram_parameter("xt", [F, B_CORE], mybir.dt.float32r, isOutput=False)
    wt = nc.declare_dram_parameter("wt", [F, M], mybir.dt.float32r, isOutput=False)
    # sel[:, 4j+i] = 1 if i == j else 0: column-selector stationaries so the
    # four per-chunk head-sum matmuls accumulate into distinct rows of one
    # [GROUP, CHUNK] PSUM tile.
    sel = nc.declare_dram_parameter(
        "sel", [M, GROUP * GROUP], mybir.dt.float32r, isOutput=False
    )
    out = nc.declare_dram_parameter(
        "out", [N_CHUNKS, CHUNK], mybir.dt.float32, isOutput=True
    )

    with tile.TileContext(nc) as tc:
        with (
            tc.tile_pool(name="wpool", bufs=1) as wpool,
            tc.tile_pool(name="xpool", bufs=3) as xpool,
            tc.tile_pool(name="spool", bufs=4) as spool,
            tc.tile_pool(name="ypool", bufs=4) as ypool,
            tc.tile_pool(name="opool", bufs=3) as opool,
            tc.tile_pool(name="zpsum", bufs=4, space="PSUM") as zpsum,
            tc.tile_pool(name="opsum", bufs=2, space="PSUM") as opsum,
        ):
            ws = wpool.tile([F, M], mybir.dt.float32r)
            nc.sync.dma_start(out=ws[:], in_=wt[:])
            sel_s = wpool.tile([M, GROUP * GROUP], mybir.dt.float32r)
            nc.sync.dma_start(out=sel_s[:], in_=sel[:])

            for st in range(N_SUPER):
                xs = xpool.tile([F, SUPER], mybir.dt.float32r)
                nc.sync.dma_start(
                    out=xs[:], in_=xt[:, st * SUPER : (st + 1) * SUPER]
                )
                # one PSUM tile accumulates GROUP chunk-sums, one per row
                o_acc = opsum.tile([GROUP, CHUNK], mybir.dt.float32)
                for j in range(CHUNKS_PER_SUPER):
                    zt = zpsum.tile([M, CHUNK], mybir.dt.float32)
                    nc.tensor.matmul(
                        zt[:],
                        lhsT=ws[:],
                        rhs=xs[:, j * CHUNK : (j + 1) * CHUNK],
                        start=True,
                        stop=True,
                    )
                    s = spool.tile([M, CHUNK], mybir.dt.float32)
                    nc.scalar.activation(
                        s[:], zt[:], mybir.ActivationFunctionType.Square
                    )
                    y = ypool.tile([M, CHUNK], mybir.dt.float32r)
                    nc.vector.scalar_tensor_tensor(
                        out=y[:],
                        in0=s[:],
                        scalar=ALPHA,
                        in1=zt[:],
                        op0=mybir.AluOpType.add,
                        op1=mybir.AluOpType.mult,
                    )
                    nc.tensor.matmul(
                        o_acc[:],
                        lhsT=sel_s[:, GROUP * j : GROUP * (j + 1)],
                        rhs=y[:],
                        start=(j == 0),
                        stop=(j == CHUNKS_PER_SUPER - 1),
                    )
                osb = opool.tile([GROUP, CHUNK], mybir.dt.float32)
                nc.scalar.copy(osb[:], o_acc[:])
                nc.sync.dma_start(
                    out=out[st * GROUP : (st + 1) * GROUP, :], in_=osb[:]
                )
    nc.finalize()
    return nc


def _run(x, W, trace=False, **run_kwargs):
    x = np.ascontiguousarray(x, dtype=np.float32)
    W = np.ascontiguousarray(W, dtype=np.float32)
    wt_np = np.ascontiguousarray(W.T)  # (128, 100)
    sel_np = np.zeros((M, GROUP * GROUP), dtype=np.float32)
    for j in range(GROUP):
        sel_np[:, GROUP * j + j] = 1.0

    in_maps = []
    for c in range(N_CORES):
        shard = x[c * B_CORE : (c + 1) * B_CORE, :]  # (32768, 128)
        xt_np = np.ascontiguousarray(shard.T)  # (128, 32768)
        in_maps.append({"xt": xt_np, "wt": wt_np, "sel": sel_np})

    nc = build_nc()
    res = run_bass_kernel_spmd(
        nc, in_maps, list(range(N_CORES)), trace=trace, **run_kwargs
    )
    outs = [res.results[c]["out"].reshape(B_CORE, 1) for c in range(N_CORES)]
    full = np.concatenate(outs, axis=0)  # (262144, 1)
    return full, res


def kernel(x, W):
    full, _ = _run(x, W)
    return full
